# revision 1
# baseline (speedup 1.0000x reference)
"""AbsPosAttention Trainium2 kernel, 8-way sharded (2 batch x 4 head-groups).

Reference (per batch b):
  q = split_heads(x @ Wq) * scale               [H, N, dk]
  k = split_heads(x @ Wk)                       [H, N, dk]
  v = split_heads(x @ Wv)                       [H, N, dv]
  qb = q + pos_embed + rel_content_bias
  out = softmax(qb @ k^T) @ v                   per head
  y = concat_heads(out) @ Wo + bo

Sharding: core c = 4*b + g computes batch b, heads {2g, 2g+1}; host sums the
4 group partials per batch, transposes (device emits y^T) and adds bo.

Per-core structure (matmuls contract over the SBUF partition dim,
out = lhsT.T @ rhs):
  A: per i-block of 512: qT/kT[128=2*dk, N] via weight-stationary chunks
     (wq/wk[c] lhsT, xt[c] moving); V[j-tile, dv] via xt-slice lhsT with
     wv[c] moving (384 cols); V evicted to v_all bf16 with a trailing ones
     column per head ([v | 1], 194-stride) for the softmax denominator.
  B: per (i-block, j-tile): both heads' logits^T land in one [128,1024]
     2-bank PSUM tile via two adjacent K=64 matmuls (row groups 0/64 run
     concurrently); one Exp activation -> pt bf16 [128,1024]; AV accumulates
     O^T unnormalized over j (av1: v rows 0..127, av2: v rows 128..191 +
     denominator row). Software-pipelined 2 j-tiles deep.
     Post-ib: den row -> reciprocal -> gpsimd partition-broadcast -> DVE
     multiplies normalize O^T into o_sb bf16, laid out as 3 contiguous
     128-row chunks covering both heads' 384 dv rows.
  C: y^T[e, i] = wo_chunk.T @ o_chunk, weight-stationary (wo bf16 lhsT,
     o_sb moving), accumulated over the 3 dv chunks; y shipped bf16.

f32r keeps full PE rate for the precision-critical q/k/logits path; the
exp output, V, O and Wo run bf16 (measured combined rel-err ~6e-3 vs the
2e-2 gate). LDWEIGHTS hides behind the PE reorder window, so matmuls
issue at stream rate; the logits pair shares the array via row packing.
"""

import numpy as np

HEADS, DIM_KEY, DIM_VALUE, DIM, N, B = 8, 64, 192, 1536, 2048, 2
SCALE = DIM_KEY**-0.5
NCORES, GROUPS, HPC = 8, 4, 2
NCH = DIM // 128  # 12 contraction chunks for the projections
NIB = N // 512  # 4 i-blocks
NJT = N // 128  # 16 j-tiles
DVC = HPC * DIM_VALUE  # 384
VH = DIM_VALUE + 1  # 193: [v | 1] per head
VHP = VH + 1  # 194: padded per-head stride in v_all
VJ = 2 * VHP  # 388: per-j-tile stride
NEC = DIM // 128  # 12 e-chunks in phase C

_cached = {}


def _install_patches():
    """Work around this walrus build's 1-sync-wait-per-instruction limit."""
    import concourse.tile as _tile
    from concourse import mybir

    def _drain_and_barrier(self, tick_clock, wait_clock):
        nc = self.nc
        probe = nc.sync.nop(nofuse=True, hint="tail_drain_waits")
        wait_clock.add_sem_waits(
            probe.ins, _tile.ScopedClock({None: tick_clock.global_clock})
        )
        si = probe.ins.sync_info
        waits = list(si.on_wait) if si and si.on_wait else []
        if len(waits) > 1:
            probe.ins.sync_info.on_wait = waits[:1]
            for w in waits[1:]:
                extra = nc.sync.nop(nofuse=True, hint="tail_drain_waits")
                esi = extra.ins.sync_info
                if esi is None:
                    extra.ins.sync_info = mybir.SyncInfo(on_wait=[w], on_update=[])
                else:
                    esi.on_wait = [w]
        nc.sync.drain()
        nc.all_engine_barrier()
        assert self.sems is not None
        popped = nc._tile_sem_poison_stack.pop()
        assert popped is self._sem_poison
        nc.clear_and_free_semaphores(list(self.sems.allocated().values()))
        nc.all_engine_barrier()

    _tile.TileContext._drain_and_barrier = _drain_and_barrier


def _split_sync_waits(nc, max_waits=1):
    from concourse import mybir

    for f in nc.m.functions:
        for bb in f.blocks:
            insts = list(bb.instructions)
            out = []
            changed = False
            for inst in insts:
                si = getattr(inst, "sync_info", None)
                if si is not None and si.on_wait and len(si.on_wait) > max_waits:
                    waits = list(si.on_wait)
                    extra, keep = waits[:-max_waits], waits[-max_waits:]
                    si.on_wait = keep
                    for i in range(0, len(extra), max_waits):
                        out.append(
                            mybir.InstNoOp(
                                name=nc.get_next_instruction_name(),
                                engine=inst.engine,
                                ins=[],
                                outs=[],
                                sync_info=mybir.SyncInfo(
                                    on_wait=extra[i : i + max_waits], on_update=[]
                                ),
                                bass_nofuse=True,
                            )
                        )
                    changed = True
                out.append(inst)
            if changed:
                bb.instructions[:] = out


def _build(split_waits=True):
    from contextlib import ExitStack

    import concourse.bass as bass
    import concourse.tile as tile
    from concourse import mybir
    from concourse.bass import ts

    _install_patches()

    f32 = mybir.dt.float32
    f32r = mybir.dt.float32r
    bf16 = mybir.dt.bfloat16
    EXP = mybir.ActivationFunctionType.Exp
    MULT = mybir.AluOpType.mult

    nc = bass.Bass()
    xt = nc.dram_tensor("xt", [DIM, N], f32r, kind="ExternalInput")
    wq = nc.dram_tensor("wq", [DIM, 128], f32r, kind="ExternalInput")
    wk = nc.dram_tensor("wk", [DIM, 128], f32r, kind="ExternalInput")
    wv = nc.dram_tensor("wv", [DIM, DVC], f32r, kind="ExternalInput")
    posb = nc.dram_tensor("posb", [128, N], f32, kind="ExternalInput")
    wo = nc.dram_tensor("wo", [DVC, DIM], bf16, kind="ExternalInput")
    y = nc.dram_tensor("y", [DIM, N], bf16, kind="ExternalOutput")

    from concourse import library_config

    with tile.TileContext(nc) as tc:
        with ExitStack() as ctx:
            sb = ctx.enter_context(tc.tile_pool(name="sb", bufs=1))
            ps = ctx.enter_context(tc.tile_pool(name="ps", bufs=1, space="PSUM"))

            # ---- persistent SBUF + input DMA (multi-queue) -------------
            wq_sb = sb.tile([128, NCH * 128], f32r, tag="wq")
            wk_sb = sb.tile([128, NCH * 128], f32r, tag="wk")
            wv_sb = sb.tile([128, NCH * DVC], f32r, tag="wv")
            posb_sb = sb.tile([128, N], f32, tag="posb")
            wo_sb = sb.tile([128, 3 * DIM], bf16, tag="wo")
            for c in range(NCH):
                nc.scalar.dma_start(wq_sb[:, ts(c, 128)], wq[ts(c, 128), :])
                nc.scalar.dma_start(wk_sb[:, ts(c, 128)], wk[ts(c, 128), :])
            for c in range(NCH):
                nc.scalar.dma_start(wv_sb[:, ts(c, DVC)], wv[ts(c, 128), :])
            nc.scalar.dma_start(posb_sb[:, 0:1024], posb[:, 0:1024])
            nc.scalar.dma_start(posb_sb[:, 1024:2048], posb[:, 1024:2048])
            for k in range(3):
                nc.scalar.dma_start(wo_sb[:, ts(k, DIM)], wo[ts(k, 128), :])

            qT = sb.tile([128, N], f32r, tag="qT")
            kT = sb.tile([128, N], f32r, tag="kT")
            v_all = sb.tile([128, NJT * VJ], bf16, tag="v_all")
            o_sb = sb.tile([128, 3 * N], bf16, tag="o_sb")

            ones_view = v_all[:].rearrange("p (j h c) -> p j h c", j=NJT, h=HPC)
            nc.vector.memset(ones_view[:, :, :, 192:193], 1.0)
            ones_t = sb.tile([128, 128], f32, tag="ones_t")
            nc.vector.memset(ones_t[:], 1.0)

            # ---- Phase A: projections ----------------------------------
            for ib in range(NIB):
                xts = []
                for c in range(NCH):
                    t = sb.tile([128, 512], f32r, name=f"xt{c}", tag=f"xt{c}", bufs=2)
                    eng = nc.sync if c % 2 == 0 else nc.gpsimd
                    eng.dma_start(t[:], xt[ts(c, 128), ts(ib, 512)])
                    xts.append(t)

                qps = ps.tile([128, 512], f32, name="qps", tag="p0")
                for c in range(NCH):
                    nc.tensor.matmul(
                        qps[:],
                        wq_sb[:, ts(c, 128)],
                        xts[c][:],
                        start=(c == 0),
                        stop=(c == NCH - 1),
                    )
                nc.vector.tensor_add(
                    qT[:, ts(ib, 512)], qps[:], posb_sb[:, ts(ib, 512)]
                )
                kps = ps.tile([128, 512], f32, name="kps", tag="p1")
                for c in range(NCH):
                    nc.tensor.matmul(
                        kps[:],
                        wk_sb[:, ts(c, 128)],
                        xts[c][:],
                        start=(c == 0),
                        stop=(c == NCH - 1),
                    )
                nc.vector.tensor_copy(kT[:, ts(ib, 512)], kps[:])

                for jl in range(4):
                    j = ib * 4 + jl
                    vps = ps.tile([128, DVC], f32, name="vps", tag="lg", bufs=2)
                    for c in range(NCH):
                        nc.tensor.matmul(
                            vps[:],
                            xts[c][:, ts(jl, 128)],
                            wv_sb[:, ts(c, DVC)],
                            start=(c == 0),
                            stop=(c == NCH - 1),
                        )
                    for h in range(HPC):
                        nc.vector.tensor_copy(
                            v_all[:, j * VJ + h * VHP : j * VJ + h * VHP + 192],
                            vps[:, ts(h, 192)],
                        )

            # ---- Phase B: attention ------------------------------------
            # lg/exp pipelined 2 j-tiles ahead of the AV accumulation.
            def lg_exp(ib, j):
                lgt = ps.tile([128, 1024], f32, name="lg", tag="lg", bufs=2)
                for h in range(HPC):
                    nc.tensor.matmul(
                        lgt[:, ts(h, 512)],
                        kT[ts(h, 64), ts(j, 128)],
                        qT[ts(h, 64), ts(ib, 512)],
                        start=True,
                        stop=True,
                    )
                pt = sb.tile([128, 1024], bf16, name="pt", tag="pt", bufs=6)
                nc.scalar.activation(pt[:], lgt[:], EXP)
                return pt

            flat = [(ib, j) for ib in range(NIB) for j in range(NJT)]

            def lg_exp_flat(idx):
                if idx < len(flat):
                    return lg_exp(*flat[idx])
                return None

            rcb = [
                sb.tile([128, 512], f32, name=f"rcb{h}", tag=f"rcb{h}", bufs=2)
                for h in range(HPC)
            ]

            pt_q = [lg_exp_flat(0), lg_exp_flat(1)]
            for ib in range(NIB):
                av1 = [
                    ps.tile([128, 512], f32, name=f"av1_{h}", tag=f"p{h}")
                    for h in range(HPC)
                ]
                av2 = [
                    ps.tile([65, 512], f32, name=f"av2_{h}", tag=f"p{2 + h}")
                    for h in range(HPC)
                ]
                for j in range(NJT):
                    pt = pt_q.pop(0)
                    pt_q.append(lg_exp_flat(ib * NJT + j + 2))
                    for h in range(HPC):
                        nc.tensor.matmul(
                            av1[h][:],
                            v_all[:, j * VJ + h * VHP : j * VJ + h * VHP + 128],
                            pt[:, ts(h, 512)],
                            start=(j == 0),
                            stop=(j == NJT - 1),
                        )
                        nc.tensor.matmul(
                            av2[h][:],
                            v_all[:, j * VJ + h * VHP + 128 : j * VJ + h * VHP + VH],
                            pt[:, ts(h, 512)],
                            start=(j == 0),
                            stop=(j == NJT - 1),
                        )
                # Raw-evict av psum to SBUF fast (unblocks av for ib+1),
                # then normalize out-of-band during the next i-block.
                raw1 = [
                    sb.tile([128, 512], f32, name=f"raw1_{h}", tag=f"raw1_{h}", bufs=2)
                    for h in range(HPC)
                ]
                raw2 = [
                    sb.tile([128, 512], f32, name=f"raw2_{h}", tag=f"raw2_{h}", bufs=2)
                    for h in range(HPC)
                ]
                for h in range(HPC):
                    nc.vector.tensor_copy(raw1[h][:], av1[h][:])
                    nc.vector.tensor_copy(raw2[h][0:65, :], av2[h][0:65, :])
                for h in range(HPC):
                    rps = ps.tile([128, 512], f32, name=f"rps{h}", tag="lg", bufs=2)
                    nc.tensor.matmul(
                        rps[:], ones_t[64:65, :], raw2[h][64:65, :],
                        start=True, stop=True,
                    )
                    nc.vector.reciprocal(rcb[h][:], rps[:])
                nc.vector.tensor_tensor(
                    o_sb[:, ib * 512 : ib * 512 + 512], raw1[0][:], rcb[0][:], MULT
                )
                nc.vector.tensor_tensor(
                    o_sb[0:64, N + ib * 512 : N + ib * 512 + 512],
                    raw2[0][0:64, :], rcb[0][0:64, :], MULT,
                )
                stg = sb.tile([128, 3 * 512], bf16, name="stg", tag="stg", bufs=2)
                nc.vector.tensor_tensor(
                    stg[0:64, 0:512], raw1[1][0:64, :], rcb[1][0:64, :], MULT
                )
                nc.vector.tensor_tensor(
                    stg[64:128, 512:1024], raw1[1][64:128, :], rcb[1][64:128, :], MULT
                )
                nc.vector.tensor_tensor(
                    stg[0:64, 1024:1536], raw2[1][0:64, :], rcb[1][0:64, :], MULT
                )
                nc.sync.dma_start(
                    o_sb[64:128, N + ib * 512 : N + ib * 512 + 512],
                    stg[0:64, 0:512],
                )
                nc.sync.dma_start(
                    o_sb[0:64, 2 * N + ib * 512 : 2 * N + ib * 512 + 512],
                    stg[64:128, 512:1024],
                )
                nc.sync.dma_start(
                    o_sb[64:128, 2 * N + ib * 512 : 2 * N + ib * 512 + 512],
                    stg[0:64, 1024:1536],
                )

            # ---- Phase C: output projection (y^T = wo^T @ O^T) ---------
            for e in range(NEC):
                if e % 2 == 0:
                    yps = [
                        ps.tile([128, 1024], f32, name=f"y{p}", tag="lg", bufs=2)
                        for p in range(2)
                    ]
                    youts = [yps[p][:, ts(i, 512)] for p in range(2) for i in range(2)]
                else:
                    yp4 = [
                        ps.tile([128, 512], f32, name=f"y{p}", tag=f"p{p}")
                        for p in range(4)
                    ]
                    youts = [t[:] for t in yp4]
                for k in range(3):
                    for ib in range(NIB):
                        nc.tensor.matmul(
                            youts[ib],
                            wo_sb[:, k * DIM + e * 128 : k * DIM + e * 128 + 128],
                            o_sb[:, k * N + ib * 512 : k * N + ib * 512 + 512],
                            start=(k == 0),
                            stop=(k == 2),
                        )
                for ib in range(NIB):
                    yo = sb.tile([128, 512], bf16, name="yo", tag="yo", bufs=4)
                    nc.vector.tensor_copy(yo[:], youts[ib])
                    nc.scalar.dma_start(y[ts(e, 128), ts(ib, 512)], yo[:])

    if split_waits:
        _split_sync_waits(nc)
    return nc


def _shard_inputs(x, Wq, Wk, Wv, Wo, pos_embed, rel_content_bias):
    import ml_dtypes

    bfloat16 = ml_dtypes.bfloat16
    in_maps = []
    xts = [np.ascontiguousarray(x[b].T) for b in range(B)]
    for c in range(NCORES):
        b, g = divmod(c, GROUPS)
        h0 = g * HPC
        wq_l = np.ascontiguousarray(Wq[:, h0 * DIM_KEY : (h0 + HPC) * DIM_KEY]) * SCALE
        wk_l = np.ascontiguousarray(Wk[:, h0 * DIM_KEY : (h0 + HPC) * DIM_KEY])
        wv_l = np.ascontiguousarray(Wv[:, h0 * DIM_VALUE : (h0 + HPC) * DIM_VALUE])
        pp = (
            pos_embed[h0 : h0 + HPC] + rel_content_bias[0, h0 : h0 + HPC]
        )  # [2, N, dk]
        posb = np.ascontiguousarray(pp.transpose(0, 2, 1)).reshape(128, N)
        wo_l = np.ascontiguousarray(
            Wo[h0 * DIM_VALUE : (h0 + HPC) * DIM_VALUE]
        ).astype(bfloat16)
        in_maps.append(
            {
                "xt": xts[b],
                "wq": wq_l.astype(np.float32),
                "wk": wk_l.astype(np.float32),
                "wv": wv_l.astype(np.float32),
                "posb": posb.astype(np.float32),
                "wo": wo_l,
            }
        )
    return in_maps


def kernel(x, Wq, Wk, Wv, Wo, bo, pos_embed, rel_content_bias, _trace=False):
    from concourse.bass_utils import run_bass_kernel_spmd

    x = np.asarray(x, np.float32)
    Wq = np.asarray(Wq, np.float32)
    Wk = np.asarray(Wk, np.float32)
    Wv = np.asarray(Wv, np.float32)
    Wo = np.asarray(Wo, np.float32)
    bo = np.asarray(bo, np.float32)
    pos_embed = np.asarray(pos_embed, np.float32)
    rel_content_bias = np.asarray(rel_content_bias, np.float32)

    if "nc" not in _cached:
        _cached["nc"] = _build()
    nc = _cached["nc"]

    in_maps = _shard_inputs(x, Wq, Wk, Wv, Wo, pos_embed, rel_content_bias)
    res = run_bass_kernel_spmd(
        nc, in_maps, core_ids=list(range(NCORES)), trace=_trace
    )
    _cached["last_result"] = res

    out = np.zeros((B, N, DIM), np.float32)
    for b in range(B):
        acc = res.results[b * GROUPS]["y"].astype(np.float32)
        for g in range(1, GROUPS):
            acc = acc + res.results[b * GROUPS + g]["y"].astype(np.float32)
        out[b] = acc.T + bo[None, :]
    return out



# revision 10
# speedup vs baseline: 1.0052x; 1.0052x over previous
"""AbsPosAttention Trainium2 kernel, 8-way sharded (2 batch x 4 head-groups).

Reference (per batch b):
  q = split_heads(x @ Wq) * scale               [H, N, dk]
  k = split_heads(x @ Wk)                       [H, N, dk]
  v = split_heads(x @ Wv)                       [H, N, dv]
  qb = q + pos_embed + rel_content_bias
  out = softmax(qb @ k^T) @ v                   per head
  y = concat_heads(out) @ Wo + bo

Sharding: core c = 4*b + g computes batch b, heads {2g, 2g+1}; host sums the
4 group partials per batch, transposes (device emits y^T) and adds bo.

Per-core structure (matmuls contract over the SBUF partition dim,
out = lhsT.T @ rhs):
  A: per i-block of 512: qT/kT[128=2*dk, N] via weight-stationary chunks
     (wq/wk[c] lhsT, xt[c] moving); V[j-tile, dv] via xt-slice lhsT with
     wv[c] moving (384 cols); V evicted to v_all bf16 with a trailing ones
     column per head ([v | 1], 194-stride) for the softmax denominator.
  B: per (i-block, j-tile): both heads' logits^T land in one [128,1024]
     2-bank PSUM tile via two adjacent K=64 matmuls (row groups 0/64 run
     concurrently); one Exp activation -> pt bf16 [128,1024]; AV accumulates
     O^T unnormalized over j (av1: v rows 0..127, av2: v rows 128..191 +
     denominator row). Software-pipelined 2 j-tiles deep.
     Post-ib: den row -> reciprocal -> gpsimd partition-broadcast -> DVE
     multiplies normalize O^T into o_sb bf16, laid out as 3 contiguous
     128-row chunks covering both heads' 384 dv rows.
  C: y^T[e, i] = wo_chunk.T @ o_chunk, weight-stationary (wo bf16 lhsT,
     o_sb moving), accumulated over the 3 dv chunks; y shipped bf16.

f32r keeps full PE rate for the precision-critical q/k/logits path; the
exp output, V, O and Wo run bf16 (measured combined rel-err ~6e-3 vs the
2e-2 gate). LDWEIGHTS hides behind the PE reorder window, so matmuls
issue at stream rate; the logits pair shares the array via row packing.
"""

import numpy as np

HEADS, DIM_KEY, DIM_VALUE, DIM, N, B = 8, 64, 192, 1536, 2048, 2
SCALE = DIM_KEY**-0.5
NCORES, GROUPS, HPC = 8, 4, 2
NCH = DIM // 128  # 12 contraction chunks for the projections
NIB = N // 512  # 4 i-blocks
NJT = N // 128  # 16 j-tiles
DVC = HPC * DIM_VALUE  # 384
VH = DIM_VALUE + 1  # 193: [v | 1] per head
VHP = VH + 1  # 194: padded per-head stride in v_all
VJ = 2 * VHP  # 388: per-j-tile stride
NEC = DIM // 128  # 12 e-chunks in phase C

_cached = {}


def _install_patches():
    """Work around this walrus build's 1-sync-wait-per-instruction limit."""
    import concourse.tile as _tile
    from concourse import mybir

    def _drain_and_barrier(self, tick_clock, wait_clock):
        nc = self.nc
        probe = nc.sync.nop(nofuse=True, hint="tail_drain_waits")
        wait_clock.add_sem_waits(
            probe.ins, _tile.ScopedClock({None: tick_clock.global_clock})
        )
        si = probe.ins.sync_info
        waits = list(si.on_wait) if si and si.on_wait else []
        if len(waits) > 1:
            probe.ins.sync_info.on_wait = waits[:1]
            for w in waits[1:]:
                extra = nc.sync.nop(nofuse=True, hint="tail_drain_waits")
                esi = extra.ins.sync_info
                if esi is None:
                    extra.ins.sync_info = mybir.SyncInfo(on_wait=[w], on_update=[])
                else:
                    esi.on_wait = [w]
        nc.sync.drain()
        nc.all_engine_barrier()
        assert self.sems is not None
        popped = nc._tile_sem_poison_stack.pop()
        assert popped is self._sem_poison
        nc.clear_and_free_semaphores(list(self.sems.allocated().values()))
        nc.all_engine_barrier()

    _tile.TileContext._drain_and_barrier = _drain_and_barrier


def _split_sync_waits(nc, max_waits=1):
    from concourse import mybir

    for f in nc.m.functions:
        for bb in f.blocks:
            insts = list(bb.instructions)
            out = []
            changed = False
            for inst in insts:
                si = getattr(inst, "sync_info", None)
                if si is not None and si.on_wait and len(si.on_wait) > max_waits:
                    waits = list(si.on_wait)
                    extra, keep = waits[:-max_waits], waits[-max_waits:]
                    si.on_wait = keep
                    for i in range(0, len(extra), max_waits):
                        out.append(
                            mybir.InstNoOp(
                                name=nc.get_next_instruction_name(),
                                engine=inst.engine,
                                ins=[],
                                outs=[],
                                sync_info=mybir.SyncInfo(
                                    on_wait=extra[i : i + max_waits], on_update=[]
                                ),
                                bass_nofuse=True,
                            )
                        )
                    changed = True
                out.append(inst)
            if changed:
                bb.instructions[:] = out


def _build(split_waits=True):
    from contextlib import ExitStack

    import concourse.bass as bass
    import concourse.tile as tile
    from concourse import mybir
    from concourse.bass import ts

    _install_patches()

    f32 = mybir.dt.float32
    f32r = mybir.dt.float32r
    bf16 = mybir.dt.bfloat16
    EXP = mybir.ActivationFunctionType.Exp
    CPY = mybir.ActivationFunctionType.Copy
    MULT = mybir.AluOpType.mult

    nc = bass.Bass()
    xt = nc.dram_tensor("xt", [DIM, N], bf16, kind="ExternalInput")
    wq = nc.dram_tensor("wq", [DIM, 128], bf16, kind="ExternalInput")
    wk = nc.dram_tensor("wk", [DIM, 128], bf16, kind="ExternalInput")
    wv = nc.dram_tensor("wv", [DIM, DVC], bf16, kind="ExternalInput")
    posb = nc.dram_tensor("posb", [128, N], f32, kind="ExternalInput")
    wo = nc.dram_tensor("wo", [DVC, DIM], bf16, kind="ExternalInput")
    y = nc.dram_tensor("y", [DIM, N], bf16, kind="ExternalOutput")

    from concourse import library_config

    with tile.TileContext(nc) as tc:
        with ExitStack() as ctx:
            sb = ctx.enter_context(tc.tile_pool(name="sb", bufs=1))
            ps = ctx.enter_context(tc.tile_pool(name="ps", bufs=1, space="PSUM"))

            # ---- persistent SBUF + input DMA (multi-queue) -------------
            # xt resident as 12 full-row chunks (bf16, 4KB partition lines)
            # across 3 queues so phase A never waits on per-block DMA.
            xt_sb = [
                sb.tile([128, N], bf16, name=f"xts{c}", tag=f"xts{c}")
                for c in range(NCH)
            ]
            xq = [nc.sync, nc.gpsimd]
            for c in range(NCH):
                xq[c % 2].dma_start(xt_sb[c][:], xt[ts(c, 128), :])
            wq_sb = sb.tile([128, NCH * 128], bf16, tag="wq")
            wk_sb = sb.tile([128, NCH * 128], bf16, tag="wk")
            wv_sb = sb.tile([128, NCH * DVC], bf16, tag="wv")
            posb_sb = sb.tile([128, N], f32, tag="posb")
            wo_sb = sb.tile([128, 3 * DIM], bf16, tag="wo")
            for c in range(NCH):
                nc.scalar.dma_start(wq_sb[:, ts(c, 128)], wq[ts(c, 128), :])
                nc.scalar.dma_start(wk_sb[:, ts(c, 128)], wk[ts(c, 128), :])
            for c in range(NCH):
                nc.scalar.dma_start(wv_sb[:, ts(c, DVC)], wv[ts(c, 128), :])
            nc.scalar.dma_start(posb_sb[:, 0:1024], posb[:, 0:1024])
            nc.scalar.dma_start(posb_sb[:, 1024:2048], posb[:, 1024:2048])
            for k in range(3):
                nc.scalar.dma_start(wo_sb[:, ts(k, DIM)], wo[ts(k, 128), :])

            qT = sb.tile([128, N], f32r, tag="qT")
            kT = sb.tile([128, N], f32r, tag="kT")
            v_all = sb.tile([128, NJT * VJ], bf16, tag="v_all")
            o_sb = sb.tile([128, 3 * N], bf16, tag="o_sb")

            ones_view = v_all[:].rearrange("p (j h c) -> p j h c", j=NJT, h=HPC)
            nc.vector.memset(ones_view[:, :, :, 192:193], 1.0)
            ones_t = sb.tile([128, 128], f32, tag="ones_t")
            nc.vector.memset(ones_t[:], 1.0)

            # ---- Phase A: projections ----------------------------------
            for ib in range(NIB):
                qps = ps.tile([128, 512], f32, name="qps", tag="p0")
                for c in range(NCH):
                    nc.tensor.matmul(
                        qps[:],
                        wq_sb[:, ts(c, 128)],
                        xt_sb[c][:, ts(ib, 512)],
                        start=(c == 0),
                        stop=(c == NCH - 1),
                    )
                nc.vector.tensor_add(
                    qT[:, ts(ib, 512)], qps[:], posb_sb[:, ts(ib, 512)]
                )
                kps = ps.tile([128, 512], f32, name="kps", tag="p1")
                for c in range(NCH):
                    nc.tensor.matmul(
                        kps[:],
                        wk_sb[:, ts(c, 128)],
                        xt_sb[c][:, ts(ib, 512)],
                        start=(c == 0),
                        stop=(c == NCH - 1),
                    )
                nc.vector.tensor_copy(kT[:, ts(ib, 512)], kps[:])

                for jl in range(4):
                    j = ib * 4 + jl
                    vps = ps.tile([128, DVC], f32, name="vps", tag="lg", bufs=2)
                    for c in range(NCH):
                        nc.tensor.matmul(
                            vps[:],
                            xt_sb[c][:, ts(j, 128)],
                            wv_sb[:, ts(c, DVC)],
                            start=(c == 0),
                            stop=(c == NCH - 1),
                        )
                    for h in range(HPC):
                        nc.vector.tensor_copy(
                            v_all[:, j * VJ + h * VHP : j * VJ + h * VHP + 192],
                            vps[:, ts(h, 192)],
                        )

            # ---- Phase B: attention ------------------------------------
            # lg/exp pipelined 2 j-tiles ahead of the AV accumulation.
            def lg_exp(ib, j):
                lgt = ps.tile([128, 1024], f32, name="lg", tag="lg", bufs=2)
                for h in range(HPC):
                    nc.tensor.matmul(
                        lgt[:, ts(h, 512)],
                        kT[ts(h, 64), ts(j, 128)],
                        qT[ts(h, 64), ts(ib, 512)],
                        start=True,
                        stop=True,
                    )
                pt = sb.tile([128, 1024], bf16, name="pt", tag="pt", bufs=6)
                nc.scalar.activation(pt[:], lgt[:], EXP)
                return pt

            flat = [(ib, j) for ib in range(NIB) for j in range(NJT)]

            def lg_exp_flat(idx):
                if idx < len(flat):
                    return lg_exp(*flat[idx])
                return None

            rcb = [
                sb.tile([128, 512], f32, name=f"rcb{h}", tag=f"rcb{h}", bufs=2)
                for h in range(HPC)
            ]

            pt_q = [lg_exp_flat(0), lg_exp_flat(1)]
            for ib in range(NIB):
                av1 = [
                    ps.tile([128, 512], f32, name=f"av1_{h}", tag=f"p{h}")
                    for h in range(HPC)
                ]
                av2 = [
                    ps.tile([65, 512], f32, name=f"av2_{h}", tag=f"p{2 + h}")
                    for h in range(HPC)
                ]
                for j in range(NJT):
                    pt = pt_q.pop(0)
                    pt_q.append(lg_exp_flat(ib * NJT + j + 2))
                    for h in range(HPC):
                        nc.tensor.matmul(
                            av1[h][:],
                            v_all[:, j * VJ + h * VHP : j * VJ + h * VHP + 128],
                            pt[:, ts(h, 512)],
                            start=(j == 0),
                            stop=(j == NJT - 1),
                        )
                        nc.tensor.matmul(
                            av2[h][:],
                            v_all[:, j * VJ + h * VHP + 128 : j * VJ + h * VHP + VH],
                            pt[:, ts(h, 512)],
                            start=(j == 0),
                            stop=(j == NJT - 1),
                        )
                # Raw-evict av psum to SBUF fast (unblocks av for ib+1),
                # then normalize out-of-band during the next i-block.
                raw1 = [
                    sb.tile([128, 512], f32, name=f"raw1_{h}", tag=f"raw1_{h}", bufs=2)
                    for h in range(HPC)
                ]
                raw2 = [
                    sb.tile([128, 512], f32, name=f"raw2_{h}", tag=f"raw2_{h}", bufs=2)
                    for h in range(HPC)
                ]
                for h in range(HPC):
                    nc.vector.tensor_copy(raw1[h][:], av1[h][:])
                    nc.scalar.activation(raw2[h][0:65, :], av2[h][0:65, :], CPY)
                for h in range(HPC):
                    # reciprocal on the single denominator row, THEN
                    # broadcast -- keeps the DVE cost at 512 elems.
                    rrow = sb.tile(
                        [1, 512], f32, name=f"rrow{h}", tag=f"rrow{h}", bufs=2
                    )
                    nc.vector.reciprocal(rrow[:], raw2[h][64:65, :])
                    rps = ps.tile([128, 512], f32, name=f"rps{h}", tag="lg", bufs=2)
                    nc.tensor.matmul(
                        rps[:], ones_t[0:1, :], rrow[:],
                        start=True, stop=True,
                    )
                    nc.vector.tensor_copy(rcb[h][:], rps[:])
                nc.vector.tensor_tensor(
                    o_sb[:, ib * 512 : ib * 512 + 512], raw1[0][:], rcb[0][:], MULT
                )
                nc.vector.tensor_tensor(
                    o_sb[0:64, N + ib * 512 : N + ib * 512 + 512],
                    raw2[0][0:64, :], rcb[0][0:64, :], MULT,
                )
                stg = sb.tile([128, 3 * 512], bf16, name="stg", tag="stg", bufs=2)
                nc.vector.tensor_tensor(
                    stg[0:64, 0:512], raw1[1][0:64, :], rcb[1][0:64, :], MULT
                )
                nc.vector.tensor_tensor(
                    stg[64:128, 512:1024], raw1[1][64:128, :], rcb[1][64:128, :], MULT
                )
                nc.vector.tensor_tensor(
                    stg[0:64, 1024:1536], raw2[1][0:64, :], rcb[1][0:64, :], MULT
                )
                nc.sync.dma_start(
                    o_sb[64:128, N + ib * 512 : N + ib * 512 + 512],
                    stg[0:64, 0:512],
                )
                nc.sync.dma_start(
                    o_sb[0:64, 2 * N + ib * 512 : 2 * N + ib * 512 + 512],
                    stg[64:128, 512:1024],
                )
                nc.sync.dma_start(
                    o_sb[64:128, 2 * N + ib * 512 : 2 * N + ib * 512 + 512],
                    stg[0:64, 1024:1536],
                )

            # ---- Phase C: output projection (y^T = wo^T @ O^T) ---------
            for e in range(NEC):
                if e % 2 == 0:
                    yps = [
                        ps.tile([128, 1024], f32, name=f"y{p}", tag="lg", bufs=2)
                        for p in range(2)
                    ]
                    youts = [yps[p][:, ts(i, 512)] for p in range(2) for i in range(2)]
                else:
                    yp4 = [
                        ps.tile([128, 512], f32, name=f"y{p}", tag=f"p{p}")
                        for p in range(4)
                    ]
                    youts = [t[:] for t in yp4]
                for k in range(3):
                    for ib in range(NIB):
                        nc.tensor.matmul(
                            youts[ib],
                            wo_sb[:, k * DIM + e * 128 : k * DIM + e * 128 + 128],
                            o_sb[:, k * N + ib * 512 : k * N + ib * 512 + 512],
                            start=(k == 0),
                            stop=(k == 2),
                        )
                for ib in range(NIB):
                    yo = sb.tile([128, 512], bf16, name="yo", tag="yo", bufs=4)
                    if ib % 2 == 0:
                        nc.vector.tensor_copy(yo[:], youts[ib])
                    else:
                        nc.scalar.activation(yo[:], youts[ib], CPY)
                    nc.sync.dma_start(y[ts(e, 128), ts(ib, 512)], yo[:])

    if split_waits:
        _split_sync_waits(nc)
    return nc


def _shard_inputs(x, Wq, Wk, Wv, Wo, pos_embed, rel_content_bias):
    import ml_dtypes

    bfloat16 = ml_dtypes.bfloat16
    in_maps = []
    xts = [np.ascontiguousarray(x[b].T).astype(bfloat16) for b in range(B)]
    for c in range(NCORES):
        b, g = divmod(c, GROUPS)
        h0 = g * HPC
        wq_l = np.ascontiguousarray(Wq[:, h0 * DIM_KEY : (h0 + HPC) * DIM_KEY]) * SCALE
        wk_l = np.ascontiguousarray(Wk[:, h0 * DIM_KEY : (h0 + HPC) * DIM_KEY])
        wv_l = np.ascontiguousarray(Wv[:, h0 * DIM_VALUE : (h0 + HPC) * DIM_VALUE])
        pp = (
            pos_embed[h0 : h0 + HPC] + rel_content_bias[0, h0 : h0 + HPC]
        )  # [2, N, dk]
        posb = np.ascontiguousarray(pp.transpose(0, 2, 1)).reshape(128, N)
        wo_l = np.ascontiguousarray(
            Wo[h0 * DIM_VALUE : (h0 + HPC) * DIM_VALUE]
        ).astype(bfloat16)
        in_maps.append(
            {
                "xt": xts[b],
                "wq": wq_l.astype(bfloat16),
                "wk": wk_l.astype(bfloat16),
                "wv": wv_l.astype(bfloat16),
                "posb": posb.astype(np.float32),
                "wo": wo_l,
            }
        )
    return in_maps


def kernel(x, Wq, Wk, Wv, Wo, bo, pos_embed, rel_content_bias, _trace=False):
    from concourse.bass_utils import run_bass_kernel_spmd

    x = np.asarray(x, np.float32)
    Wq = np.asarray(Wq, np.float32)
    Wk = np.asarray(Wk, np.float32)
    Wv = np.asarray(Wv, np.float32)
    Wo = np.asarray(Wo, np.float32)
    bo = np.asarray(bo, np.float32)
    pos_embed = np.asarray(pos_embed, np.float32)
    rel_content_bias = np.asarray(rel_content_bias, np.float32)

    if "nc" not in _cached:
        _cached["nc"] = _build()
    nc = _cached["nc"]

    in_maps = _shard_inputs(x, Wq, Wk, Wv, Wo, pos_embed, rel_content_bias)
    res = run_bass_kernel_spmd(
        nc, in_maps, core_ids=list(range(NCORES)), trace=_trace
    )
    _cached["last_result"] = res

    out = np.zeros((B, N, DIM), np.float32)
    for b in range(B):
        acc = res.results[b * GROUPS]["y"].astype(np.float32)
        for g in range(1, GROUPS):
            acc = acc + res.results[b * GROUPS + g]["y"].astype(np.float32)
        out[b] = acc.T + bo[None, :]
    return out



# revision 18
# speedup vs baseline: 1.0355x; 1.0302x over previous
"""AbsPosAttention Trainium2 kernel, 8-way sharded (2 batch x 4 head-groups).

Reference (per batch b):
  q = split_heads(x @ Wq) * scale               [H, N, dk]
  k = split_heads(x @ Wk)                       [H, N, dk]
  v = split_heads(x @ Wv)                       [H, N, dv]
  qb = q + pos_embed + rel_content_bias
  out = softmax(qb @ k^T) @ v                   per head
  y = concat_heads(out) @ Wo + bo

Sharding: core c = 4*b + g computes batch b, heads {2g, 2g+1}; host sums the
4 group partials per batch, transposes (device emits y^T) and adds bo.

Per-core structure (matmuls contract over the SBUF partition dim,
out = lhsT.T @ rhs):
  A: per i-block of 512: qT/kT[128=2*dk, N] via weight-stationary chunks
     (wq/wk[c] lhsT, xt[c] moving); V[j-tile, dv] via xt-slice lhsT with
     wv[c] moving (384 cols); V evicted to v_all bf16 with a trailing ones
     column per head ([v | 1], 194-stride) for the softmax denominator.
  B: per (i-block, j-tile): both heads' logits^T land in one [128,1024]
     2-bank PSUM tile via two adjacent K=64 matmuls (row groups 0/64 run
     concurrently); one Exp activation -> pt bf16 [128,1024]; AV accumulates
     O^T unnormalized over j (av1: v rows 0..127, av2: v rows 128..191 +
     denominator row). Software-pipelined 2 j-tiles deep.
     Post-ib: den row -> reciprocal -> gpsimd partition-broadcast -> DVE
     multiplies normalize O^T into o_sb bf16, laid out as 3 contiguous
     128-row chunks covering both heads' 384 dv rows.
  C: y^T[e, i] = wo_chunk.T @ o_chunk, weight-stationary (wo bf16 lhsT,
     o_sb moving), accumulated over the 3 dv chunks; y shipped bf16.

f32r keeps full PE rate for the precision-critical q/k/logits path; the
exp output, V, O and Wo run bf16 (measured combined rel-err ~6e-3 vs the
2e-2 gate). LDWEIGHTS hides behind the PE reorder window, so matmuls
issue at stream rate; the logits pair shares the array via row packing.
"""

import numpy as np

HEADS, DIM_KEY, DIM_VALUE, DIM, N, B = 8, 64, 192, 1536, 2048, 2
SCALE = DIM_KEY**-0.5
NCORES, GROUPS, HPC = 8, 4, 2
NCH = DIM // 128  # 12 contraction chunks for the projections
NIB = N // 512  # 4 i-blocks
NJT = N // 128  # 16 j-tiles
DVC = HPC * DIM_VALUE  # 384
VH = DIM_VALUE + 1  # 193: [v | 1] per head
VHP = VH + 1  # 194: padded per-head stride in v_all
VJ = 2 * VHP  # 388: per-j-tile stride
NEC = DIM // 128  # 12 e-chunks in phase C

_cached = {}


def _install_patches():
    """Work around this walrus build's 1-sync-wait-per-instruction limit."""
    import concourse.tile as _tile
    from concourse import mybir

    def _drain_and_barrier(self, tick_clock, wait_clock):
        nc = self.nc
        probe = nc.sync.nop(nofuse=True, hint="tail_drain_waits")
        wait_clock.add_sem_waits(
            probe.ins, _tile.ScopedClock({None: tick_clock.global_clock})
        )
        si = probe.ins.sync_info
        waits = list(si.on_wait) if si and si.on_wait else []
        if len(waits) > 1:
            probe.ins.sync_info.on_wait = waits[:1]
            for w in waits[1:]:
                extra = nc.sync.nop(nofuse=True, hint="tail_drain_waits")
                esi = extra.ins.sync_info
                if esi is None:
                    extra.ins.sync_info = mybir.SyncInfo(on_wait=[w], on_update=[])
                else:
                    esi.on_wait = [w]
        nc.sync.drain()
        nc.all_engine_barrier()
        assert self.sems is not None
        popped = nc._tile_sem_poison_stack.pop()
        assert popped is self._sem_poison
        nc.clear_and_free_semaphores(list(self.sems.allocated().values()))
        nc.all_engine_barrier()

    _tile.TileContext._drain_and_barrier = _drain_and_barrier


def _split_sync_waits(nc, max_waits=1):
    from concourse import mybir

    for f in nc.m.functions:
        for bb in f.blocks:
            insts = list(bb.instructions)
            out = []
            changed = False
            for inst in insts:
                si = getattr(inst, "sync_info", None)
                if si is not None and si.on_wait and len(si.on_wait) > max_waits:
                    waits = list(si.on_wait)
                    extra, keep = waits[:-max_waits], waits[-max_waits:]
                    si.on_wait = keep
                    for i in range(0, len(extra), max_waits):
                        out.append(
                            mybir.InstNoOp(
                                name=nc.get_next_instruction_name(),
                                engine=inst.engine,
                                ins=[],
                                outs=[],
                                sync_info=mybir.SyncInfo(
                                    on_wait=extra[i : i + max_waits], on_update=[]
                                ),
                                bass_nofuse=True,
                            )
                        )
                    changed = True
                out.append(inst)
            if changed:
                bb.instructions[:] = out


def _build(split_waits=True):
    from contextlib import ExitStack

    import concourse.bass as bass
    import concourse.tile as tile
    from concourse import mybir
    from concourse.bass import ts

    _install_patches()

    f32 = mybir.dt.float32
    f32r = mybir.dt.float32r
    bf16 = mybir.dt.bfloat16
    EXP = mybir.ActivationFunctionType.Exp
    CPY = mybir.ActivationFunctionType.Copy
    LN = mybir.ActivationFunctionType.Ln
    MULT = mybir.AluOpType.mult

    nc = bass.Bass()
    xt = nc.dram_tensor("xt", [DIM, N], bf16, kind="ExternalInput")
    wq = nc.dram_tensor("wq", [DIM, 128], bf16, kind="ExternalInput")
    wk = nc.dram_tensor("wk", [DIM, 128], bf16, kind="ExternalInput")
    wv = nc.dram_tensor("wv", [DIM, DVC], bf16, kind="ExternalInput")
    posb = nc.dram_tensor("posb", [128, N], f32, kind="ExternalInput")
    wo = nc.dram_tensor("wo", [DVC, DIM], bf16, kind="ExternalInput")
    y = nc.dram_tensor("y", [DIM, N], bf16, kind="ExternalOutput")

    from concourse import library_config

    with tile.TileContext(nc) as tc:
        with ExitStack() as ctx:
            sb = ctx.enter_context(tc.tile_pool(name="sb", bufs=1))
            ps = ctx.enter_context(tc.tile_pool(name="ps", bufs=1, space="PSUM"))

            # ---- persistent SBUF + input DMA (multi-queue) -------------
            # xt resident as 12 full-row chunks (bf16, 4KB partition lines)
            # across 3 queues so phase A never waits on per-block DMA.
            xt_sb = [
                sb.tile([128, N], bf16, name=f"xts{c}", tag=f"xts{c}")
                for c in range(NCH)
            ]
            xq = [nc.sync, nc.gpsimd]
            for c in range(NCH):
                xq[c % 2].dma_start(xt_sb[c][:], xt[ts(c, 128), :])
            wq_sb = sb.tile([128, NCH * 128], bf16, tag="wq")
            wk_sb = sb.tile([128, NCH * 128], bf16, tag="wk")
            wv_sb = sb.tile([128, NCH * DVC], bf16, tag="wv")
            posb_sb = sb.tile([128, N], f32, tag="posb")
            wo_sb = sb.tile([128, 3 * DIM], bf16, tag="wo")
            for c in range(NCH):
                nc.scalar.dma_start(wq_sb[:, ts(c, 128)], wq[ts(c, 128), :])
                nc.scalar.dma_start(wk_sb[:, ts(c, 128)], wk[ts(c, 128), :])
            for c in range(NCH):
                nc.scalar.dma_start(wv_sb[:, ts(c, DVC)], wv[ts(c, 128), :])
            nc.scalar.dma_start(posb_sb[:, 0:1024], posb[:, 0:1024])
            nc.scalar.dma_start(posb_sb[:, 1024:2048], posb[:, 1024:2048])
            for k in range(3):
                nc.scalar.dma_start(wo_sb[:, ts(k, DIM)], wo[ts(k, 128), :])

            qT = sb.tile([128, N], f32r, tag="qT")
            kT = sb.tile([128, N], f32r, tag="kT")
            v_all = sb.tile([128, NJT * VJ], bf16, tag="v_all")
            o_sb = sb.tile([128, 3 * N], bf16, tag="o_sb")

            ones_view = v_all[:].rearrange("p (j h c) -> p j h c", j=NJT, h=HPC)
            nc.vector.memset(ones_view[:, :, :, 192:193], 1.0)
            ones_t = sb.tile([128, 128], f32, tag="ones_t")
            nc.vector.memset(ones_t[:], 1.0)

            # ---- Phase A: projections ----------------------------------
            for ib in range(NIB):
                qps = ps.tile([128, 512], f32, name="qps", tag="p0")
                for c in range(NCH):
                    nc.tensor.matmul(
                        qps[:],
                        wq_sb[:, ts(c, 128)],
                        xt_sb[c][:, ts(ib, 512)],
                        start=(c == 0),
                        stop=(c == NCH - 1),
                    )
                nc.vector.tensor_add(
                    qT[:, ts(ib, 512)], qps[:], posb_sb[:, ts(ib, 512)]
                )
                kps = ps.tile([128, 512], f32, name="kps", tag="p1")
                for c in range(NCH):
                    nc.tensor.matmul(
                        kps[:],
                        wk_sb[:, ts(c, 128)],
                        xt_sb[c][:, ts(ib, 512)],
                        start=(c == 0),
                        stop=(c == NCH - 1),
                    )
                nc.vector.tensor_copy(kT[:, ts(ib, 512)], kps[:])

                for jl in range(4):
                    j = ib * 4 + jl
                    vps = ps.tile([128, DVC], f32, name="vps", tag="lg", bufs=2)
                    for c in range(NCH):
                        nc.tensor.matmul(
                            vps[:],
                            xt_sb[c][:, ts(j, 128)],
                            wv_sb[:, ts(c, DVC)],
                            start=(c == 0),
                            stop=(c == NCH - 1),
                        )
                    for h in range(HPC):
                        nc.vector.tensor_copy(
                            v_all[:, j * VJ + h * VHP : j * VJ + h * VHP + 192],
                            vps[:, ts(h, 192)],
                        )

            # ---- Phase B: attention ------------------------------------
            # lg/exp pipelined 2 j-tiles ahead of the AV accumulation.
            def lg_exp(ib, j):
                lgt = ps.tile([128, 1024], f32, name="lg", tag="lg", bufs=2)
                for h in range(HPC):
                    nc.tensor.matmul(
                        lgt[:, ts(h, 512)],
                        kT[ts(h, 64), ts(j, 128)],
                        qT[ts(h, 64), ts(ib, 512)],
                        start=True,
                        stop=True,
                    )
                pt = sb.tile([128, 1024], bf16, name="pt", tag="pt", bufs=6)
                nc.scalar.activation(pt[:], lgt[:], EXP)
                return pt

            flat = [(ib, j) for ib in range(NIB) for j in range(NJT)]

            def lg_exp_flat(idx):
                if idx < len(flat):
                    return lg_exp(*flat[idx])
                return None

            rcb = [
                sb.tile([128, 512], f32, name=f"rcb{h}", tag=f"rcb{h}", bufs=2)
                for h in range(HPC)
            ]

            pt_q = [lg_exp_flat(0), lg_exp_flat(1)]
            for ib in range(NIB):
                av1 = [
                    ps.tile([128, 512], f32, name=f"av1_{h}", tag=f"p{h}")
                    for h in range(HPC)
                ]
                av2 = [
                    ps.tile([65, 512], f32, name=f"av2_{h}", tag=f"p{2 + h}")
                    for h in range(HPC)
                ]
                for j in range(NJT):
                    pt = pt_q.pop(0)
                    pt_q.append(lg_exp_flat(ib * NJT + j + 2))
                    for h in range(HPC):
                        nc.tensor.matmul(
                            av1[h][:],
                            v_all[:, j * VJ + h * VHP : j * VJ + h * VHP + 128],
                            pt[:, ts(h, 512)],
                            start=(j == 0),
                            stop=(j == NJT - 1),
                        )
                        nc.tensor.matmul(
                            av2[h][:],
                            v_all[:, j * VJ + h * VHP + 128 : j * VJ + h * VHP + VH],
                            pt[:, ts(h, 512)],
                            start=(j == 0),
                            stop=(j == NJT - 1),
                        )
                # Raw-evict av psum to SBUF fast (unblocks av for ib+1),
                # then normalize out-of-band during the next i-block.
                raw1 = [
                    sb.tile([128, 512], f32, name=f"raw1_{h}", tag=f"raw1_{h}", bufs=2)
                    for h in range(HPC)
                ]
                raw2 = [
                    sb.tile([128, 512], f32, name=f"raw2_{h}", tag=f"raw2_{h}", bufs=2)
                    for h in range(HPC)
                ]
                for h in range(HPC):
                    nc.vector.tensor_copy(raw1[h][:], av1[h][:])
                for h in range(HPC):
                    # Denominator reciprocal as exp(-ln Z) on the scalar
                    # engine straight off the PSUM row; broadcast on gpsimd.
                    # Keeps the boundary off the lg-psum rotation and the
                    # 3.3us single-lane DVE reciprocal off the DVE queue.
                    rln = sb.tile(
                        [1, 512], f32, name=f"rln{h}", tag=f"rln{h}", bufs=2
                    )
                    rrow = sb.tile(
                        [1, 512], f32, name=f"rrow{h}", tag=f"rrow{h}", bufs=2
                    )
                    nc.scalar.activation(rln[:], av2[h][64:65, :], LN)
                    nc.scalar.activation(raw2[h][0:64, :], av2[h][0:64, :], CPY)
                    nc.scalar.activation(rrow[:], rln[:], EXP, scale=-1.0)
                    src = rrow[:]
                    bc = bass.AP(
                        tensor=src.tensor,
                        offset=src.offset,
                        ap=[list(src.ap[0]), [0, 128]] + list(src.ap[1:]),
                    )
                    nc.gpsimd.dma_start(rcb[h][:], bc)
                nc.vector.tensor_tensor(
                    o_sb[:, ib * 512 : ib * 512 + 512], raw1[0][:], rcb[0][:], MULT
                )
                nc.vector.tensor_tensor(
                    o_sb[0:64, N + ib * 512 : N + ib * 512 + 512],
                    raw2[0][0:64, :], rcb[0][0:64, :], MULT,
                )
                stg = sb.tile([128, 3 * 512], bf16, name="stg", tag="stg", bufs=2)
                nc.vector.tensor_tensor(
                    stg[0:64, 0:512], raw1[1][0:64, :], rcb[1][0:64, :], MULT
                )
                nc.vector.tensor_tensor(
                    stg[64:128, 512:1024], raw1[1][64:128, :], rcb[1][64:128, :], MULT
                )
                nc.vector.tensor_tensor(
                    stg[0:64, 1024:1536], raw2[1][0:64, :], rcb[1][0:64, :], MULT
                )
                nc.sync.dma_start(
                    o_sb[64:128, N + ib * 512 : N + ib * 512 + 512],
                    stg[0:64, 0:512],
                )
                nc.sync.dma_start(
                    o_sb[0:64, 2 * N + ib * 512 : 2 * N + ib * 512 + 512],
                    stg[64:128, 512:1024],
                )
                nc.sync.dma_start(
                    o_sb[64:128, 2 * N + ib * 512 : 2 * N + ib * 512 + 512],
                    stg[0:64, 1024:1536],
                )

            # ---- Phase C: output projection (y^T = wo^T @ O^T) ---------
            for e in range(NEC):
                if e % 2 == 0:
                    yps = [
                        ps.tile([128, 1024], f32, name=f"y{p}", tag="lg", bufs=2)
                        for p in range(2)
                    ]
                    youts = [yps[p][:, ts(i, 512)] for p in range(2) for i in range(2)]
                else:
                    yp4 = [
                        ps.tile([128, 512], f32, name=f"y{p}", tag=f"p{p}")
                        for p in range(4)
                    ]
                    youts = [t[:] for t in yp4]
                for k in range(3):
                    for ib in range(NIB):
                        nc.tensor.matmul(
                            youts[ib],
                            wo_sb[:, k * DIM + e * 128 : k * DIM + e * 128 + 128],
                            o_sb[:, k * N + ib * 512 : k * N + ib * 512 + 512],
                            start=(k == 0),
                            stop=(k == 2),
                        )
                for ib in range(NIB):
                    yo = sb.tile([128, 512], bf16, name="yo", tag="yo", bufs=4)
                    if ib % 2 == 0:
                        nc.vector.tensor_copy(yo[:], youts[ib])
                    else:
                        nc.scalar.activation(yo[:], youts[ib], CPY)
                    nc.sync.dma_start(y[ts(e, 128), ts(ib, 512)], yo[:])

    if split_waits:
        _split_sync_waits(nc)
    return nc


def _shard_inputs(x, Wq, Wk, Wv, Wo, pos_embed, rel_content_bias):
    import ml_dtypes

    bfloat16 = ml_dtypes.bfloat16
    in_maps = []
    xts = [np.ascontiguousarray(x[b].T).astype(bfloat16) for b in range(B)]
    for c in range(NCORES):
        b, g = divmod(c, GROUPS)
        h0 = g * HPC
        wq_l = np.ascontiguousarray(Wq[:, h0 * DIM_KEY : (h0 + HPC) * DIM_KEY]) * SCALE
        wk_l = np.ascontiguousarray(Wk[:, h0 * DIM_KEY : (h0 + HPC) * DIM_KEY])
        wv_l = np.ascontiguousarray(Wv[:, h0 * DIM_VALUE : (h0 + HPC) * DIM_VALUE])
        pp = (
            pos_embed[h0 : h0 + HPC] + rel_content_bias[0, h0 : h0 + HPC]
        )  # [2, N, dk]
        posb = np.ascontiguousarray(pp.transpose(0, 2, 1)).reshape(128, N)
        wo_l = np.ascontiguousarray(
            Wo[h0 * DIM_VALUE : (h0 + HPC) * DIM_VALUE]
        ).astype(bfloat16)
        in_maps.append(
            {
                "xt": xts[b],
                "wq": wq_l.astype(bfloat16),
                "wk": wk_l.astype(bfloat16),
                "wv": wv_l.astype(bfloat16),
                "posb": posb.astype(np.float32),
                "wo": wo_l,
            }
        )
    return in_maps


def kernel(x, Wq, Wk, Wv, Wo, bo, pos_embed, rel_content_bias, _trace=False):
    from concourse.bass_utils import run_bass_kernel_spmd

    x = np.asarray(x, np.float32)
    Wq = np.asarray(Wq, np.float32)
    Wk = np.asarray(Wk, np.float32)
    Wv = np.asarray(Wv, np.float32)
    Wo = np.asarray(Wo, np.float32)
    bo = np.asarray(bo, np.float32)
    pos_embed = np.asarray(pos_embed, np.float32)
    rel_content_bias = np.asarray(rel_content_bias, np.float32)

    if "nc" not in _cached:
        _cached["nc"] = _build()
    nc = _cached["nc"]

    in_maps = _shard_inputs(x, Wq, Wk, Wv, Wo, pos_embed, rel_content_bias)
    res = run_bass_kernel_spmd(
        nc, in_maps, core_ids=list(range(NCORES)), trace=_trace
    )
    _cached["last_result"] = res

    out = np.zeros((B, N, DIM), np.float32)
    for b in range(B):
        acc = res.results[b * GROUPS]["y"].astype(np.float32)
        for g in range(1, GROUPS):
            acc = acc + res.results[b * GROUPS + g]["y"].astype(np.float32)
        out[b] = acc.T + bo[None, :]
    return out



# revision 22
# speedup vs baseline: 1.0357x; 1.0001x over previous
"""AbsPosAttention Trainium2 kernel, 8-way sharded (2 batch x 4 head-groups).

Reference (per batch b):
  q = split_heads(x @ Wq) * scale               [H, N, dk]
  k = split_heads(x @ Wk)                       [H, N, dk]
  v = split_heads(x @ Wv)                       [H, N, dv]
  qb = q + pos_embed + rel_content_bias
  out = softmax(qb @ k^T) @ v                   per head
  y = concat_heads(out) @ Wo + bo

Sharding: core c = 4*b + g computes batch b, heads {2g, 2g+1}; host sums the
4 group partials per batch, transposes (device emits y^T) and adds bo.

Per-core structure (matmuls contract over the SBUF partition dim,
out = lhsT.T @ rhs):
  A: per i-block of 512: qT/kT[128=2*dk, N] via weight-stationary chunks
     (wq/wk[c] lhsT, xt[c] moving); V[j-tile, dv] via xt-slice lhsT with
     wv[c] moving (384 cols); V evicted to v_all bf16 with a trailing ones
     column per head ([v | 1], 194-stride) for the softmax denominator.
  B: per (i-block, j-tile): both heads' logits^T land in one [128,1024]
     2-bank PSUM tile via two adjacent K=64 matmuls (row groups 0/64 run
     concurrently); one Exp activation -> pt bf16 [128,1024]; AV accumulates
     O^T unnormalized over j (av1: v rows 0..127, av2: v rows 128..191 +
     denominator row). Software-pipelined 2 j-tiles deep.
     Post-ib: den row -> reciprocal -> gpsimd partition-broadcast -> DVE
     multiplies normalize O^T into o_sb bf16, laid out as 3 contiguous
     128-row chunks covering both heads' 384 dv rows.
  C: y^T[e, i] = wo_chunk.T @ o_chunk, weight-stationary (wo bf16 lhsT,
     o_sb moving), accumulated over the 3 dv chunks; y shipped bf16.

f32r keeps full PE rate for the precision-critical q/k/logits path; the
exp output, V, O and Wo run bf16 (measured combined rel-err ~6e-3 vs the
2e-2 gate). LDWEIGHTS hides behind the PE reorder window, so matmuls
issue at stream rate; the logits pair shares the array via row packing.
"""

import numpy as np

HEADS, DIM_KEY, DIM_VALUE, DIM, N, B = 8, 64, 192, 1536, 2048, 2
SCALE = DIM_KEY**-0.5
NCORES, GROUPS, HPC = 8, 4, 2
NCH = DIM // 128  # 12 contraction chunks for the projections
NIB = N // 512  # 4 i-blocks
NJT = N // 128  # 16 j-tiles
DVC = HPC * DIM_VALUE  # 384
VH = DIM_VALUE + 1  # 193: [v | 1] per head
VHP = VH + 1  # 194: padded per-head stride in v_all
VJ = 2 * VHP  # 388: per-j-tile stride
NEC = DIM // 128  # 12 e-chunks in phase C

_cached = {}


def _install_patches():
    """Work around this walrus build's 1-sync-wait-per-instruction limit."""
    import concourse.tile as _tile
    from concourse import mybir

    def _drain_and_barrier(self, tick_clock, wait_clock):
        nc = self.nc
        probe = nc.sync.nop(nofuse=True, hint="tail_drain_waits")
        wait_clock.add_sem_waits(
            probe.ins, _tile.ScopedClock({None: tick_clock.global_clock})
        )
        si = probe.ins.sync_info
        waits = list(si.on_wait) if si and si.on_wait else []
        if len(waits) > 1:
            probe.ins.sync_info.on_wait = waits[:1]
            for w in waits[1:]:
                extra = nc.sync.nop(nofuse=True, hint="tail_drain_waits")
                esi = extra.ins.sync_info
                if esi is None:
                    extra.ins.sync_info = mybir.SyncInfo(on_wait=[w], on_update=[])
                else:
                    esi.on_wait = [w]
        nc.sync.drain()
        nc.all_engine_barrier()
        assert self.sems is not None
        popped = nc._tile_sem_poison_stack.pop()
        assert popped is self._sem_poison
        nc.clear_and_free_semaphores(list(self.sems.allocated().values()))
        nc.all_engine_barrier()

    _tile.TileContext._drain_and_barrier = _drain_and_barrier


def _split_sync_waits(nc, max_waits=1):
    from concourse import mybir

    for f in nc.m.functions:
        for bb in f.blocks:
            insts = list(bb.instructions)
            out = []
            changed = False
            for inst in insts:
                si = getattr(inst, "sync_info", None)
                if si is not None and si.on_wait and len(si.on_wait) > max_waits:
                    waits = list(si.on_wait)
                    extra, keep = waits[:-max_waits], waits[-max_waits:]
                    si.on_wait = keep
                    for i in range(0, len(extra), max_waits):
                        out.append(
                            mybir.InstNoOp(
                                name=nc.get_next_instruction_name(),
                                engine=inst.engine,
                                ins=[],
                                outs=[],
                                sync_info=mybir.SyncInfo(
                                    on_wait=extra[i : i + max_waits], on_update=[]
                                ),
                                bass_nofuse=True,
                            )
                        )
                    changed = True
                out.append(inst)
            if changed:
                bb.instructions[:] = out


def _build(split_waits=True):
    from contextlib import ExitStack

    import concourse.bass as bass
    import concourse.tile as tile
    from concourse import mybir
    from concourse.bass import ts

    _install_patches()

    f32 = mybir.dt.float32
    f32r = mybir.dt.float32r
    bf16 = mybir.dt.bfloat16
    EXP = mybir.ActivationFunctionType.Exp
    CPY = mybir.ActivationFunctionType.Copy
    LN = mybir.ActivationFunctionType.Ln
    MULT = mybir.AluOpType.mult

    nc = bass.Bass()
    xt = nc.dram_tensor("xt", [DIM, N], bf16, kind="ExternalInput")
    # wq/wk/wv pre-rearranged on the host into SBUF layout so each loads
    # with ONE big-line DMA (3KB/9KB partition lines vs 256B fragments).
    wq = nc.dram_tensor("wq", [128, NCH * 128], bf16, kind="ExternalInput")
    wk = nc.dram_tensor("wk", [128, NCH * 128], bf16, kind="ExternalInput")
    wv = nc.dram_tensor("wv", [128, NCH * DVC], bf16, kind="ExternalInput")
    posb = nc.dram_tensor("posb", [128, N], f32, kind="ExternalInput")
    wo = nc.dram_tensor("wo", [DVC, DIM], bf16, kind="ExternalInput")
    y = nc.dram_tensor("y", [DIM, N], bf16, kind="ExternalOutput")

    from concourse import library_config

    with tile.TileContext(nc) as tc:
        with ExitStack() as ctx:
            sb = ctx.enter_context(tc.tile_pool(name="sb", bufs=1))
            ps = ctx.enter_context(tc.tile_pool(name="ps", bufs=1, space="PSUM"))

            # ---- persistent SBUF + input DMA (multi-queue) -------------
            # xt resident as 12 full-row chunks (bf16, 4KB partition lines)
            # across 3 queues so phase A never waits on per-block DMA.
            xt_sb = [
                sb.tile([128, N], bf16, name=f"xts{c}", tag=f"xts{c}")
                for c in range(NCH)
            ]
            xq = [nc.sync, nc.gpsimd]
            for c in range(NCH):
                xq[c % 2].dma_start(xt_sb[c][:], xt[ts(c, 128), :])
            wq_sb = sb.tile([128, NCH * 128], bf16, tag="wq")
            wk_sb = sb.tile([128, NCH * 128], bf16, tag="wk")
            wv_sb = sb.tile([128, NCH * DVC], bf16, tag="wv")
            posb_sb = sb.tile([128, N], f32, tag="posb")
            wo_sb = sb.tile([128, 3 * DIM], bf16, tag="wo")
            nc.scalar.dma_start(wq_sb[:], wq[:, :])
            nc.scalar.dma_start(wk_sb[:], wk[:, :])
            nc.scalar.dma_start(wv_sb[:], wv[:, :])
            nc.scalar.dma_start(posb_sb[:, 0:1024], posb[:, 0:1024])
            nc.scalar.dma_start(posb_sb[:, 1024:2048], posb[:, 1024:2048])
            for k in range(3):
                nc.scalar.dma_start(wo_sb[:, ts(k, DIM)], wo[ts(k, 128), :])

            qT = sb.tile([128, N], f32r, tag="qT")
            kT = sb.tile([128, N], f32r, tag="kT")
            v_all = sb.tile([128, NJT * VJ], bf16, tag="v_all")
            o_sb = sb.tile([128, 3 * N], bf16, tag="o_sb")

            ones_view = v_all[:].rearrange("p (j h c) -> p j h c", j=NJT, h=HPC)
            nc.vector.memset(ones_view[:, :, :, 192:193], 1.0)
            ones_t = sb.tile([128, 128], f32, tag="ones_t")
            nc.vector.memset(ones_t[:], 1.0)

            # ---- Phase A: projections ----------------------------------
            for ib in range(NIB):
                qps = ps.tile([128, 512], f32, name="qps", tag="p0")
                for c in range(NCH):
                    nc.tensor.matmul(
                        qps[:],
                        wq_sb[:, ts(c, 128)],
                        xt_sb[c][:, ts(ib, 512)],
                        start=(c == 0),
                        stop=(c == NCH - 1),
                    )
                nc.vector.tensor_add(
                    qT[:, ts(ib, 512)], qps[:], posb_sb[:, ts(ib, 512)]
                )
                kps = ps.tile([128, 512], f32, name="kps", tag="p1")
                for c in range(NCH):
                    nc.tensor.matmul(
                        kps[:],
                        wk_sb[:, ts(c, 128)],
                        xt_sb[c][:, ts(ib, 512)],
                        start=(c == 0),
                        stop=(c == NCH - 1),
                    )
                nc.vector.tensor_copy(kT[:, ts(ib, 512)], kps[:])

                for jl in range(4):
                    j = ib * 4 + jl
                    vps = ps.tile([128, DVC], f32, name="vps", tag="lg", bufs=2)
                    for c in range(NCH):
                        nc.tensor.matmul(
                            vps[:],
                            xt_sb[c][:, ts(j, 128)],
                            wv_sb[:, ts(c, DVC)],
                            start=(c == 0),
                            stop=(c == NCH - 1),
                        )
                    for h in range(HPC):
                        nc.vector.tensor_copy(
                            v_all[:, j * VJ + h * VHP : j * VJ + h * VHP + 192],
                            vps[:, ts(h, 192)],
                        )

            # ---- Phase B: attention ------------------------------------
            # lg/exp pipelined 2 j-tiles ahead of the AV accumulation.
            def lg_exp(ib, j):
                lgt = ps.tile([128, 1024], f32, name="lg", tag="lg", bufs=2)
                for h in range(HPC):
                    nc.tensor.matmul(
                        lgt[:, ts(h, 512)],
                        kT[ts(h, 64), ts(j, 128)],
                        qT[ts(h, 64), ts(ib, 512)],
                        start=True,
                        stop=True,
                    )
                pt = sb.tile([128, 1024], bf16, name="pt", tag="pt", bufs=6)
                nc.scalar.activation(pt[:], lgt[:], EXP)
                return pt

            flat = [(ib, j) for ib in range(NIB) for j in range(NJT)]

            def lg_exp_flat(idx):
                if idx < len(flat):
                    return lg_exp(*flat[idx])
                return None

            rcb = [
                sb.tile([128, 512], f32, name=f"rcb{h}", tag=f"rcb{h}", bufs=2)
                for h in range(HPC)
            ]

            pt_q = [lg_exp_flat(0), lg_exp_flat(1)]
            for ib in range(NIB):
                av1 = [
                    ps.tile([128, 512], f32, name=f"av1_{h}", tag=f"p{h}")
                    for h in range(HPC)
                ]
                av2 = [
                    ps.tile([65, 512], f32, name=f"av2_{h}", tag=f"p{2 + h}")
                    for h in range(HPC)
                ]
                for j in range(NJT):
                    pt = pt_q.pop(0)
                    pt_q.append(lg_exp_flat(ib * NJT + j + 2))
                    for h in range(HPC):
                        nc.tensor.matmul(
                            av1[h][:],
                            v_all[:, j * VJ + h * VHP : j * VJ + h * VHP + 128],
                            pt[:, ts(h, 512)],
                            start=(j == 0),
                            stop=(j == NJT - 1),
                        )
                        nc.tensor.matmul(
                            av2[h][:],
                            v_all[:, j * VJ + h * VHP + 128 : j * VJ + h * VHP + VH],
                            pt[:, ts(h, 512)],
                            start=(j == 0),
                            stop=(j == NJT - 1),
                        )
                # Raw-evict av psum to SBUF fast (unblocks av for ib+1),
                # then normalize out-of-band during the next i-block.
                raw1 = [
                    sb.tile([128, 512], f32, name=f"raw1_{h}", tag=f"raw1_{h}", bufs=2)
                    for h in range(HPC)
                ]
                raw2 = [
                    sb.tile([128, 512], f32, name=f"raw2_{h}", tag=f"raw2_{h}", bufs=2)
                    for h in range(HPC)
                ]
                for h in range(HPC):
                    nc.vector.tensor_copy(raw1[h][:], av1[h][:])
                for h in range(HPC):
                    # Denominator reciprocal as exp(-ln Z) on the scalar
                    # engine straight off the PSUM row; broadcast on gpsimd.
                    # Keeps the boundary off the lg-psum rotation and the
                    # 3.3us single-lane DVE reciprocal off the DVE queue.
                    rln = sb.tile(
                        [1, 512], f32, name=f"rln{h}", tag=f"rln{h}", bufs=2
                    )
                    rrow = sb.tile(
                        [1, 512], f32, name=f"rrow{h}", tag=f"rrow{h}", bufs=2
                    )
                    nc.scalar.activation(rln[:], av2[h][64:65, :], LN)
                    nc.scalar.activation(raw2[h][0:64, :], av2[h][0:64, :], CPY)
                    nc.scalar.activation(rrow[:], rln[:], EXP, scale=-1.0)
                    src = rrow[:]
                    bc = bass.AP(
                        tensor=src.tensor,
                        offset=src.offset,
                        ap=[list(src.ap[0]), [0, 128]] + list(src.ap[1:]),
                    )
                    nc.gpsimd.dma_start(rcb[h][:], bc)
                # normalize on gpsimd: keeps o_sb production off the DVE
                # queue so phase C's psum evictions can't starve it.
                nc.gpsimd.tensor_tensor(
                    o_sb[:, ib * 512 : ib * 512 + 512], raw1[0][:], rcb[0][:], MULT
                )
                nc.gpsimd.tensor_tensor(
                    o_sb[0:64, N + ib * 512 : N + ib * 512 + 512],
                    raw2[0][0:64, :], rcb[0][0:64, :], MULT,
                )
                stg = sb.tile([128, 3 * 512], bf16, name="stg", tag="stg", bufs=2)
                nc.gpsimd.tensor_tensor(
                    stg[0:64, 0:512], raw1[1][0:64, :], rcb[1][0:64, :], MULT
                )
                nc.gpsimd.tensor_tensor(
                    stg[64:128, 512:1024], raw1[1][64:128, :], rcb[1][64:128, :], MULT
                )
                nc.gpsimd.tensor_tensor(
                    stg[0:64, 1024:1536], raw2[1][0:64, :], rcb[1][0:64, :], MULT
                )
                nc.sync.dma_start(
                    o_sb[64:128, N + ib * 512 : N + ib * 512 + 512],
                    stg[0:64, 0:512],
                )
                nc.sync.dma_start(
                    o_sb[0:64, 2 * N + ib * 512 : 2 * N + ib * 512 + 512],
                    stg[64:128, 512:1024],
                )
                nc.sync.dma_start(
                    o_sb[64:128, 2 * N + ib * 512 : 2 * N + ib * 512 + 512],
                    stg[0:64, 1024:1536],
                )

            # ---- Phase C: output projection (y^T = wo^T @ O^T) ---------
            for e in range(NEC):
                if e % 2 == 0:
                    yps = [
                        ps.tile([128, 1024], f32, name=f"y{p}", tag="lg", bufs=2)
                        for p in range(2)
                    ]
                    youts = [yps[p][:, ts(i, 512)] for p in range(2) for i in range(2)]
                else:
                    yp4 = [
                        ps.tile([128, 512], f32, name=f"y{p}", tag=f"p{p}")
                        for p in range(4)
                    ]
                    youts = [t[:] for t in yp4]
                for k in range(3):
                    for ib in range(NIB):
                        nc.tensor.matmul(
                            youts[ib],
                            wo_sb[:, k * DIM + e * 128 : k * DIM + e * 128 + 128],
                            o_sb[:, k * N + ib * 512 : k * N + ib * 512 + 512],
                            start=(k == 0),
                            stop=(k == 2),
                        )
                for ib in range(NIB):
                    yo = sb.tile([128, 512], bf16, name="yo", tag="yo", bufs=4)
                    if ib % 2 == 0:
                        nc.vector.tensor_copy(yo[:], youts[ib])
                    else:
                        nc.scalar.activation(yo[:], youts[ib], CPY)
                    nc.sync.dma_start(y[ts(e, 128), ts(ib, 512)], yo[:])

    if split_waits:
        _split_sync_waits(nc)
    return nc


def _shard_inputs(x, Wq, Wk, Wv, Wo, pos_embed, rel_content_bias):
    import ml_dtypes

    bfloat16 = ml_dtypes.bfloat16
    in_maps = []
    xts = [np.ascontiguousarray(x[b].T).astype(bfloat16) for b in range(B)]
    for c in range(NCORES):
        b, g = divmod(c, GROUPS)
        h0 = g * HPC
        wq_l = np.ascontiguousarray(Wq[:, h0 * DIM_KEY : (h0 + HPC) * DIM_KEY]) * SCALE
        wk_l = np.ascontiguousarray(Wk[:, h0 * DIM_KEY : (h0 + HPC) * DIM_KEY])
        wv_l = np.ascontiguousarray(Wv[:, h0 * DIM_VALUE : (h0 + HPC) * DIM_VALUE])
        pp = (
            pos_embed[h0 : h0 + HPC] + rel_content_bias[0, h0 : h0 + HPC]
        )  # [2, N, dk]
        posb = np.ascontiguousarray(pp.transpose(0, 2, 1)).reshape(128, N)
        wo_l = np.ascontiguousarray(
            Wo[h0 * DIM_VALUE : (h0 + HPC) * DIM_VALUE]
        ).astype(bfloat16)
        # rearrange to SBUF layout: [128, nch*free], block c = rows 128c..
        wq_r = wq_l.reshape(NCH, 128, 128).transpose(1, 0, 2).reshape(128, -1)
        wk_r = wk_l.reshape(NCH, 128, 128).transpose(1, 0, 2).reshape(128, -1)
        wv_r = wv_l.reshape(NCH, 128, DVC).transpose(1, 0, 2).reshape(128, -1)
        in_maps.append(
            {
                "xt": xts[b],
                "wq": np.ascontiguousarray(wq_r).astype(bfloat16),
                "wk": np.ascontiguousarray(wk_r).astype(bfloat16),
                "wv": np.ascontiguousarray(wv_r).astype(bfloat16),
                "posb": posb.astype(np.float32),
                "wo": wo_l,
            }
        )
    return in_maps


def kernel(x, Wq, Wk, Wv, Wo, bo, pos_embed, rel_content_bias, _trace=False):
    from concourse.bass_utils import run_bass_kernel_spmd

    x = np.asarray(x, np.float32)
    Wq = np.asarray(Wq, np.float32)
    Wk = np.asarray(Wk, np.float32)
    Wv = np.asarray(Wv, np.float32)
    Wo = np.asarray(Wo, np.float32)
    bo = np.asarray(bo, np.float32)
    pos_embed = np.asarray(pos_embed, np.float32)
    rel_content_bias = np.asarray(rel_content_bias, np.float32)

    if "nc" not in _cached:
        _cached["nc"] = _build()
    nc = _cached["nc"]

    in_maps = _shard_inputs(x, Wq, Wk, Wv, Wo, pos_embed, rel_content_bias)
    res = run_bass_kernel_spmd(
        nc, in_maps, core_ids=list(range(NCORES)), trace=_trace
    )
    _cached["last_result"] = res

    out = np.zeros((B, N, DIM), np.float32)
    for b in range(B):
        acc = res.results[b * GROUPS]["y"].astype(np.float32)
        for g in range(1, GROUPS):
            acc = acc + res.results[b * GROUPS + g]["y"].astype(np.float32)
        out[b] = acc.T + bo[None, :]
    return out

